# revision 49
# baseline (speedup 1.0000x reference)
"""Trainium2 Bass kernel for nn_Choquet_Integral.

Reformulation: the Choquet integral (sort + successive diffs + FM lattice
gather + einsum) equals a Mobius-transform contraction over subset minima:

    y[b, h] = sum_{T subset of {0..7}, T nonempty} mHat[T, h] * min_{i in T} x_b[i]

where mHat is the Mobius transform of the fuzzy measure FM (host-computed,
255 x 8). Subset minima are produced with min(a,b) = (a + b - |a - b|)/2 in a
3-level balanced cascade, so everything becomes constant-matrix matmuls (PE)
interleaved with elementwise |.| (ACT/DVE). No sort, no gather.

Stages per sample n (b = the 512 d-columns, free dim):
  Z[0:8]   = x rows (DMA)
  S1 (PE): D2 = A1^T Z[0:8]            -> |.| -> Z[8:12]
  S2 (PE): D4 = A2^T Z[0:12]           -> |.| -> Z[12:30]
  S3 (PE): D8 = A3^T Z[0:30] (225 rows)-> |.| -> W8a/W8b
  y0 (PE): y  = C030^T Z[0:30]
  S4 (PE): y += C8a^T W8a + C8b^T W8b  (PSUM accumulation)
Then per 16 samples: LayerNorm over (H, D) + PReLU, batched on [128, 512]
tiles (per-sample stats via a block-diagonal ones matmul), DMA out.

Sharding: data-parallel over N across the 8 NeuronCores (256 samples each).
"""

import sys

for _p in ("/opt/trn_rl_repo", "/root/.axon_site/_ro/trn_rl_repo"):
    if _p not in sys.path:
        sys.path.append(_p)

import numpy as np

import concourse.bass as bass
import concourse.bacc as bacc
import concourse.tile as tile
from concourse import mybir
from concourse.tile_rust import add_dep_helper
from concourse.bass_utils import run_bass_kernel_spmd

N, S, D, H = 2048, 8, 512, 8
NCORES = 8
NPC = N // NCORES  # samples per core
LN_EPS = 1e-5
F32 = mybir.dt.float32
F32R = mybir.dt.float32r

NZ = 255  # Z feature rows: 8 x | 4 |d2| | 18 |d4| | 225 |d8|
# 2-sample-paired on-chip Z tile: x(A,B)@0:16, R2(A,B)@32:40, R4(A,B)@64:100
ZROWS = 100


# --------------------------------------------------------------------------
# Host-side constant matrices
# --------------------------------------------------------------------------
def _build_structure():
    """FM-independent pieces: A1 [8,4], A2 [12,18], A3 [30,225], and the
    linear forms of every subset minimum over the 255-dim Z vector."""

    def v_x(i):
        v = np.zeros(NZ)
        v[i] = 1.0
        return v

    def e(row):
        v = np.zeros(NZ)
        v[row] = 1.0
        return v

    # relu convention: min(a, b) = a - relu(a - b); row e(.) holds relu(diff)
    m2 = [v_x(2 * p) - e(8 + p) for p in range(4)]

    def P(p, a):  # pair p value for local mask a in {1,2,3}
        return (v_x(2 * p), v_x(2 * p + 1), m2[p])[a - 1]

    m4 = {0: {}, 1: {}}
    d4rows = {0: {}, 1: {}}
    for side in range(2):
        p0, p1 = (0, 1) if side == 0 else (2, 3)
        for t in range(1, 16):
            a, b = t & 3, t >> 2
            if b == 0:
                m4[side][t] = P(p0, a)
            elif a == 0:
                m4[side][t] = P(p1, b)
            else:
                d4rows[side][(a, b)] = P(p0, a) - P(p1, b)
                m4[side][t] = P(p0, a) - e(12 + 9 * side + 3 * (a - 1) + (b - 1))

    d8rows = {}
    minT = {}
    for T in range(1, 256):
        t, u = T & 15, T >> 4
        if u == 0:
            minT[T] = m4[0][t]
        elif t == 0:
            minT[T] = m4[1][u]
        else:
            d8rows[(t, u)] = m4[0][t] - m4[1][u]
            minT[T] = m4[0][t] - e(30 + 15 * (t - 1) + (u - 1))

    A1 = np.zeros((8, 4))
    for p in range(4):
        A1[2 * p, p] = 1.0
        A1[2 * p + 1, p] = -1.0

    A2 = np.zeros((12, 18))
    for side in range(2):
        for a in range(1, 4):
            for b in range(1, 4):
                A2[:, 9 * side + 3 * (a - 1) + (b - 1)] = d4rows[side][(a, b)][:12]

    A3 = np.zeros((30, 225))
    for t in range(1, 16):
        for u in range(1, 16):
            A3[:, 15 * (t - 1) + (u - 1)] = d8rows[(t, u)][:30]

    return A1, A2, A3, minT


_A1, _A2, _A3, _MINT = _build_structure()


def _mobius(FM):
    """mHat[T, h], T in [0, 255]; mu(mask) = FM[mask-1], mu(0) = 0."""
    mh = np.zeros((256, H), np.float64)
    mh[1:] = FM.astype(np.float64)
    for b in range(8):
        bit = 1 << b
        idx = np.arange(256)
        hi = idx[(idx & bit) != 0]
        mh[hi] -= mh[hi ^ bit]
    return mh


def _host_matrices(FM):
    mh = _mobius(FM)
    C = np.zeros((NZ, H))
    for T in range(1, 256):
        C += np.outer(_MINT[T], mh[T])
    f = np.float32

    # Samples are processed in PAIRS sharing one Z tile [100, D]:
    #   sample A rows: x@0:8,  R2@32:36, R4@64:82
    #   sample B rows: x@8:16, R2@36:40, R4@82:100
    # (engine ops may change partition base between in/out but only at
    # 32-aligned bases; f32r matmuls must write PSUM at base 0)
    def zscatter2(M, par):
        out = np.zeros((ZROWS, M.shape[1]))
        o = 8 * par
        out[o : o + 8] = M[0:8]
        out[32 + 4 * par : 36 + 4 * par] = M[8:12]
        out[64 + 18 * par : 82 + 18 * par] = M[12:30]
        return out

    # S1 pair matrix: [16, 8] block-diag of A1
    A1q = np.zeros((16, 8))
    A1q[0:8, 0:4] = _A1
    A1q[8:16, 4:8] = _A1

    # S2 pair matrix: [40, 36]; K rows = x(A,B)@0:16 + R2(A,B)@32:40 -> slice
    A2q = np.zeros((40, 36))
    A2q[0:8, 0:18] = _A2[0:8]
    A2q[32:36, 0:18] = _A2[8:12]
    A2q[8:16, 18:36] = _A2[0:8]
    A2q[36:40, 18:36] = _A2[8:12]

    # f32r matmuls must write PSUM at partition base 0, so per-sample y
    # outputs (8 rows) are emitted as full-bank M=128 matmuls: 16 slot
    # variants with the C columns placed at columns 8k..8k+8 of zeros.
    def slotted(Cpart, rows=None):
        Kr = Cpart.shape[0]
        out = np.zeros((16, Kr, 128), f)
        for k in range(16):
            out[k, :, 8 * k : 8 * k + 8] = Cpart
        return out

    mats = {
        "a1": A1q.astype(f),
        "a2": A2q.astype(f),
        "g16": _g16(),
    }
    for par in range(2):
        A3p = zscatter2(_A3, par)
        mats[f"a3a{par}"] = A3p[:, 0:128].astype(f)
        mats[f"a3b{par}"] = A3p[:, 128:225].astype(f)
    c030v = np.zeros((16, ZROWS, 128), f)
    for k in range(16):
        c030v[k][:, 8 * k : 8 * k + 8] = zscatter2(C[0:30, :], k % 2)
    mats["c030v"] = c030v
    mats["c8av"] = slotted(C[30:158, :])
    mats["c8bv"] = slotted(C[158:255, :])
    return mats


def _g16():
    """Block-diagonal ones [128, 128]: per-sample (8-row group) sum replicator."""
    g = np.zeros((128, 128), np.float32)
    for k in range(16):
        g[8 * k : 8 * k + 8, 8 * k : 8 * k + 8] = 1.0
    return g


# cpack column layout: every constant packed into one [128, CP] f32 tensor so
# the whole preamble is a single DMA (keeps the drain sync-wait count low).
_CPCOLS = {
    "a1": (0, 8, 16),
    "a2": (8, 36, 40),
    "a3a0": (44, 128, ZROWS),
    "a3a1": (172, 128, ZROWS),
    "a3b0": (300, 97, ZROWS),
    "a3b1": (397, 97, ZROWS),
    "c030v": (494, 128, ZROWS),  # 16 slots of 128
    "c8av": (2542, 128, 128),
    "c8bv": (4590, 128, 97),
    "g16": (6638, 128, 128),
    "lnw": (6766, D, 128),
    "lnb": (7278, D, 128),
    "pre": (7790, 1, 128),
    "eps": (7791, 1, 128),
}
CP = 7792


def _pack_consts(mats, lnw, lnb, pre_w):
    cp = np.zeros((128, CP), np.float32)

    def put(name, arr, slot=None):
        c0, w, rows = _CPCOLS[name]
        if slot is not None:
            c0 += 128 * slot
        cp[: arr.shape[0], c0 : c0 + arr.shape[1]] = arr

    put("a1", mats["a1"])
    put("a2", mats["a2"])
    for par in range(2):
        put(f"a3a{par}", mats[f"a3a{par}"])
        put(f"a3b{par}", mats[f"a3b{par}"])
    for k in range(16):
        put("c030v", mats["c030v"][k], slot=k)
        put("c8av", mats["c8av"][k], slot=k)
        put("c8bv", mats["c8bv"][k], slot=k)
    put("g16", _g16())
    put("lnw", lnw)
    put("lnb", lnb)
    cp[:, _CPCOLS["pre"][0]] = pre_w
    cp[:, _CPCOLS["eps"][0]] = LN_EPS
    return cp


# --------------------------------------------------------------------------
# Bass module
# --------------------------------------------------------------------------
def build_module(npc=NPC, mm_dtype=F32R):
    nc = bacc.Bacc("TRN2", target_bir_lowering=False, debug=False)

    x_in = nc.dram_tensor("x", [npc, S, D], mm_dtype, kind="ExternalInput").ap()
    y_out = nc.dram_tensor("y", [npc, H, D], F32, kind="ExternalOutput").ap()

    cpack = nc.dram_tensor("cpack", [128, CP], mm_dtype, kind="ExternalInput").ap()

    AluOp = mybir.AluOpType
    Act = mybir.ActivationFunctionType

    def mm(out, lhsT, rhs, **kw):
        nc.tensor.matmul(out, lhsT, rhs, **kw)

    # ---- persistent SBUF constants + Z buffers ----
    cpk = nc.alloc_sbuf_tensor("cpk", [128, CP], mm_dtype).ap()

    def cslice(name, slot=None, bitcast=None):
        c0, w, rows = _CPCOLS[name]
        if slot is not None:
            c0 += 128 * slot
        ap = cpk[0:rows, c0 : c0 + w]
        return ap.bitcast(bitcast) if bitcast is not None else ap

    ct = {
        "a1": cslice("a1"),
        "a2": cslice("a2"),
        "a3a0": cslice("a3a0"),
        "a3a1": cslice("a3a1"),
        "a3b0": cslice("a3b0"),
        "a3b1": cslice("a3b1"),
        "g16": cslice("g16", bitcast=F32),
        "lnw": cslice("lnw", bitcast=F32),
        "lnb": cslice("lnb", bitcast=F32),
    }
    for k in range(16):
        ct[f"c030v{k}"] = cslice("c030v", slot=k)
        ct[f"c8av{k}"] = cslice("c8av", slot=k)
        ct[f"c8bv{k}"] = cslice("c8bv", slot=k)
    pre = cslice("pre", bitcast=F32)
    eps = cslice("eps", bitcast=F32)
    NZB = 6
    z_bufs = [
        nc.alloc_sbuf_tensor(f"zbuf{i}", [ZROWS, D], mm_dtype).ap() for i in range(NZB)
    ]
    # pair-wide S3 PSUM tensors, double-buffered: two banks each (columns
    # 0:512 = sample A, 512:1024 = sample B) so one relu op drains both
    # samples.  NOTE: with d24+y16 this uses all 8 PSUM banks.
    p8a_pairs = [nc.alloc_psum_tensor(f"p8apair{i}", [128, 2 * D], F32).ap() for i in range(1)]
    p8b_pairs = [nc.alloc_psum_tensor(f"p8bpair{i}", [97, 2 * D], F32).ap() for i in range(1)]

    # Preamble TileContext: one const DMA + Z-buffer zeroing; its exit barrier
    # fully separates these deps from the main loop.
    with tile.TileContext(nc) as tc0:
        nc.sync.dma_start(out=cpk, in_=cpack)
        for zb in z_bufs:
            nc.gpsimd.memset(zb[:, :].bitcast(F32), 0.0)

    with tile.TileContext(nc) as tc:
        with (
            tc.tile_pool(name="wpool", bufs=4) as wpool,
            tc.tile_pool(name="lnpool", bufs=2) as lnpool,
            tc.tile_pool(name="ps_small", bufs=2, space="PSUM") as ps_small,
            tc.tile_pool(name="ps_big", bufs=1, space="PSUM") as ps_big,
            tc.tile_pool(name="ps_y", bufs=2, space="PSUM") as ps_y,
        ):
            for blk in range(npc // 16):
                # one full PSUM bank accumulates y for 16 samples (8 rows each)
                y16 = ps_y.tile([128, D], F32, tag="y16")
                for j in range(8):  # pairs of samples
                    n0 = blk * 16 + 2 * j
                    z = z_bufs[(blk * 8 + j) % NZB]
                    nc.gpsimd.dma_start(
                        out=z[0:16, :],
                        in_=x_in[n0 : n0 + 2].rearrange("n s d -> (n s) d"),
                    )

                    # paired S1 + S2 diffs in one PSUM tile at base 0 (f32r
                    # matmuls must write base 0); relu ops write the Z rows
                    # cross-base (32-aligned bases only).
                    d24 = ps_small.tile([36, D], F32, tag="d24")
                    mm(d24[0:8, :], ct["a1"], z[0:16, :])
                    nc.vector.tensor_scalar(
                        out=z[32:40, :], in0=d24[0:8, :], scalar1=0.0, scalar2=None,
                        op0=AluOp.max,
                    )
                    mm(d24[0:36, :], ct["a2"], z[0:40, :])
                    nc.scalar.activation(out=z[64:100, :], in_=d24[0:36, :], func=Act.Relu)

                    # software-pipeline within the pair: all S3 matmuls and
                    # the (fused, pair-wide) relus first, then relu-independent
                    # c030v matmuls, then the relu-consuming y accumulations.
                    p8a_pair = p8a_pairs[j % len(p8a_pairs)]
                    p8b_pair = p8b_pairs[j % len(p8b_pairs)]
                    w8a = wpool.tile([128, 2 * D], mm_dtype, tag="w8a")
                    w8b = wpool.tile([97, 2 * D], mm_dtype, tag="w8b")
                    for par in range(2):
                        cs = slice(par * D, par * D + D)
                        mm(p8a_pair[:, cs], ct[f"a3a{par}"], z[:])
                        mm(p8b_pair[:, cs], ct[f"a3b{par}"], z[:])
                    nc.scalar.activation(out=w8a[:], in_=p8a_pair[:, :], func=Act.Relu)
                    nc.vector.tensor_scalar(
                        out=w8b[:], in0=p8b_pair[:, :], scalar1=0.0, scalar2=None,
                        op0=AluOp.max,
                    )
                    for par in range(2):
                        k = 2 * j + par
                        mm(y16[:], ct[f"c030v{k}"], z[:], start=(k == 0), stop=False)
                    for par in range(2):
                        k = 2 * j + par
                        cs = slice(par * D, par * D + D)
                        mm(y16[:], ct[f"c8av{k}"], w8a[:, cs], start=False, stop=False)
                        mm(y16[:], ct[f"c8bv{k}"], w8b[:, cs], start=False, stop=(k == 15))

                # ---- LayerNorm over (H, D) per sample + PReLU, batched ----
                r = lnpool.tile([128, 2], F32, tag="r")
                sq = lnpool.tile([128, D], F32, tag="sq")
                nc.scalar.activation(
                    out=sq[:], in_=y16[:], func=Act.Copy, accum_out=r[:, 0:1]
                )
                nc.scalar.activation(
                    out=sq[:], in_=y16[:], func=Act.Square, accum_out=r[:, 1:2]
                )
                sp = ps_small.tile([128, 2], F32, tag="d24")
                nc.tensor.matmul(sp[:], ct["g16"], r[:])

                mean = lnpool.tile([128, 1], F32, tag="mean")
                var = lnpool.tile([128, 1], F32, tag="var")
                rstd = lnpool.tile([128, 1], F32, tag="rstd")
                nc.vector.tensor_scalar(
                    out=mean[:], in0=sp[:, 0:1], scalar1=1.0 / (H * D), scalar2=None,
                    op0=AluOp.mult,
                )
                # var = sp[:,1]/HD - mean^2  (guard: compute mean^2 first)
                m2t = lnpool.tile([128, 1], F32, tag="m2t")
                nc.vector.tensor_tensor(
                    out=m2t[:], in0=mean[:], in1=mean[:], op=AluOp.mult
                )
                nc.vector.scalar_tensor_tensor(
                    out=var[:], in0=sp[:, 1:2], scalar=1.0 / (H * D),
                    in1=m2t[:], op0=AluOp.mult, op1=AluOp.subtract,
                )
                # rstd = 1/sqrt(var + eps)
                nc.scalar.activation(out=rstd[:], in_=var[:], func=Act.Sqrt, bias=eps)
                nc.vector.reciprocal(out=rstd[:], in_=rstd[:])

                t1 = lnpool.tile([128, D], F32, tag="t1")
                nc.vector.tensor_scalar(
                    out=t1[:], in0=y16[:], scalar1=mean[:], scalar2=rstd[:],
                    op0=AluOp.subtract, op1=AluOp.mult,
                )
                t2 = lnpool.tile([128, D], F32, tag="t2")
                nc.gpsimd.tensor_tensor(out=t2[:], in0=t1[:], in1=ct["lnw"], op=AluOp.mult)
                nc.gpsimd.tensor_tensor(out=t2[:], in0=t2[:], in1=ct["lnb"], op=AluOp.add)
                # prelu: out = max(t2, 0) + pre * min(t2, 0)
                u = lnpool.tile([128, D], F32, tag="u")
                nc.vector.tensor_scalar(
                    out=u[:], in0=t2[:], scalar1=0.0, scalar2=pre,
                    op0=AluOp.min, op1=AluOp.mult,
                )
                o16 = lnpool.tile([128, D], F32, tag="o16")
                nc.vector.scalar_tensor_tensor(
                    out=o16[:], in0=t2[:], scalar=0.0, in1=u[:],
                    op0=AluOp.max, op1=AluOp.add,
                )
                nc.gpsimd.dma_start(
                    out=y_out[blk * 16 : blk * 16 + 16].rearrange("n h d -> (n h) d"),
                    in_=o16[:],
                )

    nc.compile()
    return nc


# --------------------------------------------------------------------------
# Entry point
# --------------------------------------------------------------------------
_CACHED = {}


def _get_module():
    if "nc" not in _CACHED:
        _CACHED["nc"] = build_module()
    return _CACHED["nc"]


def host_feeds(FM, ln_weight, ln_bias, prelu_w):
    mats = _host_matrices(np.asarray(FM, np.float64))
    lnw = np.tile(np.asarray(ln_weight, np.float32).reshape(1, H, D), (16, 1, 1)).reshape(128, D)
    lnb = np.tile(np.asarray(ln_bias, np.float32).reshape(1, H, D), (16, 1, 1)).reshape(128, D)
    return _pack_consts(mats, lnw, lnb, float(np.asarray(prelu_w).reshape(-1)[0]))


def kernel(x, FM, ln_weight, ln_bias, prelu_w):
    x = np.ascontiguousarray(np.asarray(x, np.float32))
    cpack = np.ascontiguousarray(host_feeds(FM, ln_weight, ln_bias, prelu_w))

    nc = _get_module()
    in_maps = []
    for c in range(NCORES):
        in_maps.append(
            {"x": np.ascontiguousarray(x[c * NPC : (c + 1) * NPC]), "cpack": cpack}
        )

    res = run_bass_kernel_spmd(nc, in_maps, core_ids=list(range(NCORES)))
    out = np.concatenate([r["y"] for r in res.results], axis=0)
    return out.astype(np.float32)


if __name__ == "__main__":
    # smoke-test build
    nc = build_module(npc=16)
    print("module built ok")


# revision 50
# speedup vs baseline: 1.0005x; 1.0005x over previous
"""Trainium2 Bass kernel for nn_Choquet_Integral.

Reformulation: the Choquet integral (sort + successive diffs + FM lattice
gather + einsum) equals a Mobius-transform contraction over subset minima:

    y[b, h] = sum_{T subset of {0..7}, T nonempty} mHat[T, h] * min_{i in T} x_b[i]

where mHat is the Mobius transform of the fuzzy measure FM (host-computed,
255 x 8). Subset minima are produced with min(a,b) = (a + b - |a - b|)/2 in a
3-level balanced cascade, so everything becomes constant-matrix matmuls (PE)
interleaved with elementwise |.| (ACT/DVE). No sort, no gather.

Stages per sample n (b = the 512 d-columns, free dim):
  Z[0:8]   = x rows (DMA)
  S1 (PE): D2 = A1^T Z[0:8]            -> |.| -> Z[8:12]
  S2 (PE): D4 = A2^T Z[0:12]           -> |.| -> Z[12:30]
  S3 (PE): D8 = A3^T Z[0:30] (225 rows)-> |.| -> W8a/W8b
  y0 (PE): y  = C030^T Z[0:30]
  S4 (PE): y += C8a^T W8a + C8b^T W8b  (PSUM accumulation)
Then per 16 samples: LayerNorm over (H, D) + PReLU, batched on [128, 512]
tiles (per-sample stats via a block-diagonal ones matmul), DMA out.

Sharding: data-parallel over N across the 8 NeuronCores (256 samples each).
"""

import sys

for _p in ("/opt/trn_rl_repo", "/root/.axon_site/_ro/trn_rl_repo"):
    if _p not in sys.path:
        sys.path.append(_p)

import numpy as np

import concourse.bass as bass
import concourse.bacc as bacc
import concourse.tile as tile
from concourse import mybir
from concourse.tile_rust import add_dep_helper
from concourse.bass_utils import run_bass_kernel_spmd

N, S, D, H = 2048, 8, 512, 8
NCORES = 8
NPC = N // NCORES  # samples per core
LN_EPS = 1e-5
F32 = mybir.dt.float32
F32R = mybir.dt.float32r

NZ = 255  # Z feature rows: 8 x | 4 |d2| | 18 |d4| | 225 |d8|
# 2-sample-paired on-chip Z tile: x(A,B)@0:16, R2(A,B)@32:40, R4(A,B)@64:100
ZROWS = 100


# --------------------------------------------------------------------------
# Host-side constant matrices
# --------------------------------------------------------------------------
def _build_structure():
    """FM-independent pieces: A1 [8,4], A2 [12,18], A3 [30,225], and the
    linear forms of every subset minimum over the 255-dim Z vector."""

    def v_x(i):
        v = np.zeros(NZ)
        v[i] = 1.0
        return v

    def e(row):
        v = np.zeros(NZ)
        v[row] = 1.0
        return v

    # relu convention: min(a, b) = a - relu(a - b); row e(.) holds relu(diff)
    m2 = [v_x(2 * p) - e(8 + p) for p in range(4)]

    def P(p, a):  # pair p value for local mask a in {1,2,3}
        return (v_x(2 * p), v_x(2 * p + 1), m2[p])[a - 1]

    m4 = {0: {}, 1: {}}
    d4rows = {0: {}, 1: {}}
    for side in range(2):
        p0, p1 = (0, 1) if side == 0 else (2, 3)
        for t in range(1, 16):
            a, b = t & 3, t >> 2
            if b == 0:
                m4[side][t] = P(p0, a)
            elif a == 0:
                m4[side][t] = P(p1, b)
            else:
                d4rows[side][(a, b)] = P(p0, a) - P(p1, b)
                m4[side][t] = P(p0, a) - e(12 + 9 * side + 3 * (a - 1) + (b - 1))

    d8rows = {}
    minT = {}
    for T in range(1, 256):
        t, u = T & 15, T >> 4
        if u == 0:
            minT[T] = m4[0][t]
        elif t == 0:
            minT[T] = m4[1][u]
        else:
            d8rows[(t, u)] = m4[0][t] - m4[1][u]
            minT[T] = m4[0][t] - e(30 + 15 * (t - 1) + (u - 1))

    A1 = np.zeros((8, 4))
    for p in range(4):
        A1[2 * p, p] = 1.0
        A1[2 * p + 1, p] = -1.0

    A2 = np.zeros((12, 18))
    for side in range(2):
        for a in range(1, 4):
            for b in range(1, 4):
                A2[:, 9 * side + 3 * (a - 1) + (b - 1)] = d4rows[side][(a, b)][:12]

    A3 = np.zeros((30, 225))
    for t in range(1, 16):
        for u in range(1, 16):
            A3[:, 15 * (t - 1) + (u - 1)] = d8rows[(t, u)][:30]

    return A1, A2, A3, minT


_A1, _A2, _A3, _MINT = _build_structure()


def _mobius(FM):
    """mHat[T, h], T in [0, 255]; mu(mask) = FM[mask-1], mu(0) = 0."""
    mh = np.zeros((256, H), np.float64)
    mh[1:] = FM.astype(np.float64)
    for b in range(8):
        bit = 1 << b
        idx = np.arange(256)
        hi = idx[(idx & bit) != 0]
        mh[hi] -= mh[hi ^ bit]
    return mh


def _host_matrices(FM):
    mh = _mobius(FM)
    C = np.zeros((NZ, H))
    for T in range(1, 256):
        C += np.outer(_MINT[T], mh[T])
    f = np.float32

    # Samples are processed in PAIRS sharing one Z tile [100, D]:
    #   sample A rows: x@0:8,  R2@32:36, R4@64:82
    #   sample B rows: x@8:16, R2@36:40, R4@82:100
    # (engine ops may change partition base between in/out but only at
    # 32-aligned bases; f32r matmuls must write PSUM at base 0)
    def zscatter2(M, par):
        out = np.zeros((ZROWS, M.shape[1]))
        o = 8 * par
        out[o : o + 8] = M[0:8]
        out[32 + 4 * par : 36 + 4 * par] = M[8:12]
        out[64 + 18 * par : 82 + 18 * par] = M[12:30]
        return out

    # S1 pair matrix: [16, 8] block-diag of A1
    A1q = np.zeros((16, 8))
    A1q[0:8, 0:4] = _A1
    A1q[8:16, 4:8] = _A1

    # S2 pair matrix: [40, 36]; K rows = x(A,B)@0:16 + R2(A,B)@32:40 -> slice
    A2q = np.zeros((40, 36))
    A2q[0:8, 0:18] = _A2[0:8]
    A2q[32:36, 0:18] = _A2[8:12]
    A2q[8:16, 18:36] = _A2[0:8]
    A2q[36:40, 18:36] = _A2[8:12]

    # f32r matmuls must write PSUM at partition base 0, so per-sample y
    # outputs (8 rows) are emitted as full-bank M=128 matmuls: 16 slot
    # variants with the C columns placed at columns 8k..8k+8 of zeros.
    def slotted(Cpart, rows=None):
        Kr = Cpart.shape[0]
        out = np.zeros((16, Kr, 128), f)
        for k in range(16):
            out[k, :, 8 * k : 8 * k + 8] = Cpart
        return out

    mats = {
        "a1": A1q.astype(f),
        "a2": A2q.astype(f),
        "g16": _g16(),
    }
    for par in range(2):
        A3p = zscatter2(_A3, par)
        mats[f"a3a{par}"] = A3p[:, 0:128].astype(f)
        mats[f"a3b{par}"] = A3p[:, 128:225].astype(f)
    c030v = np.zeros((16, ZROWS, 128), f)
    for k in range(16):
        c030v[k][:, 8 * k : 8 * k + 8] = zscatter2(C[0:30, :], k % 2)
    mats["c030v"] = c030v
    mats["c8av"] = slotted(C[30:158, :])
    mats["c8bv"] = slotted(C[158:255, :])
    return mats


def _g16():
    """Block-diagonal ones [128, 128]: per-sample (8-row group) sum replicator."""
    g = np.zeros((128, 128), np.float32)
    for k in range(16):
        g[8 * k : 8 * k + 8, 8 * k : 8 * k + 8] = 1.0
    return g


# cpack column layout: every constant packed into one [128, CP] f32 tensor so
# the whole preamble is a single DMA (keeps the drain sync-wait count low).
_CPCOLS = {
    "a1": (0, 8, 16),
    "a2": (8, 36, 40),
    "a3a0": (44, 128, ZROWS),
    "a3a1": (172, 128, ZROWS),
    "a3b0": (300, 97, ZROWS),
    "a3b1": (397, 97, ZROWS),
    "c030v": (494, 128, ZROWS),  # 16 slots of 128
    "c8av": (2542, 128, 128),
    "c8bv": (4590, 128, 97),
    "g16": (6638, 128, 128),
    "lnw": (6766, D, 128),
    "lnb": (7278, D, 128),
    "pre": (7790, 1, 128),
    "eps": (7791, 1, 128),
}
CP = 7792


def _pack_consts(mats, lnw, lnb, pre_w):
    cp = np.zeros((128, CP), np.float32)

    def put(name, arr, slot=None):
        c0, w, rows = _CPCOLS[name]
        if slot is not None:
            c0 += 128 * slot
        cp[: arr.shape[0], c0 : c0 + arr.shape[1]] = arr

    put("a1", mats["a1"])
    put("a2", mats["a2"])
    for par in range(2):
        put(f"a3a{par}", mats[f"a3a{par}"])
        put(f"a3b{par}", mats[f"a3b{par}"])
    for k in range(16):
        put("c030v", mats["c030v"][k], slot=k)
        put("c8av", mats["c8av"][k], slot=k)
        put("c8bv", mats["c8bv"][k], slot=k)
    put("g16", _g16())
    put("lnw", lnw)
    put("lnb", lnb)
    cp[:, _CPCOLS["pre"][0]] = pre_w
    cp[:, _CPCOLS["eps"][0]] = LN_EPS
    return cp


# --------------------------------------------------------------------------
# Bass module
# --------------------------------------------------------------------------
def build_module(npc=NPC, mm_dtype=F32R):
    nc = bacc.Bacc("TRN2", target_bir_lowering=False, debug=False)

    x_in = nc.dram_tensor("x", [npc, S, D], mm_dtype, kind="ExternalInput").ap()
    y_out = nc.dram_tensor("y", [npc, H, D], F32, kind="ExternalOutput").ap()

    cpack = nc.dram_tensor("cpack", [128, CP], mm_dtype, kind="ExternalInput").ap()

    AluOp = mybir.AluOpType
    Act = mybir.ActivationFunctionType

    def mm(out, lhsT, rhs, **kw):
        nc.tensor.matmul(out, lhsT, rhs, **kw)

    # ---- persistent SBUF constants + Z buffers ----
    cpk = nc.alloc_sbuf_tensor("cpk", [128, CP], mm_dtype).ap()

    def cslice(name, slot=None, bitcast=None):
        c0, w, rows = _CPCOLS[name]
        if slot is not None:
            c0 += 128 * slot
        ap = cpk[0:rows, c0 : c0 + w]
        return ap.bitcast(bitcast) if bitcast is not None else ap

    ct = {
        "a1": cslice("a1"),
        "a2": cslice("a2"),
        "a3a0": cslice("a3a0"),
        "a3a1": cslice("a3a1"),
        "a3b0": cslice("a3b0"),
        "a3b1": cslice("a3b1"),
        "g16": cslice("g16", bitcast=F32),
        "lnw": cslice("lnw", bitcast=F32),
        "lnb": cslice("lnb", bitcast=F32),
    }
    for k in range(16):
        ct[f"c030v{k}"] = cslice("c030v", slot=k)
        ct[f"c8av{k}"] = cslice("c8av", slot=k)
        ct[f"c8bv{k}"] = cslice("c8bv", slot=k)
    pre = cslice("pre", bitcast=F32)
    eps = cslice("eps", bitcast=F32)
    NZB = 8
    z_bufs = [
        nc.alloc_sbuf_tensor(f"zbuf{i}", [ZROWS, D], mm_dtype).ap() for i in range(NZB)
    ]
    # pair-wide S3 PSUM tensors, double-buffered: two banks each (columns
    # 0:512 = sample A, 512:1024 = sample B) so one relu op drains both
    # samples.  NOTE: with d24+y16 this uses all 8 PSUM banks.
    p8a_pairs = [nc.alloc_psum_tensor(f"p8apair{i}", [128, 2 * D], F32).ap() for i in range(1)]
    p8b_pairs = [nc.alloc_psum_tensor(f"p8bpair{i}", [97, 2 * D], F32).ap() for i in range(1)]

    # Preamble TileContext: one const DMA + Z-buffer zeroing; its exit barrier
    # fully separates these deps from the main loop.
    with tile.TileContext(nc) as tc0:
        nc.sync.dma_start(out=cpk, in_=cpack)
        for zb in z_bufs:
            nc.gpsimd.memset(zb[:, :].bitcast(F32), 0.0)

    with tile.TileContext(nc) as tc:
        with (
            tc.tile_pool(name="wpool", bufs=4) as wpool,
            tc.tile_pool(name="lnpool", bufs=3) as lnpool,
            tc.tile_pool(name="ps_small", bufs=2, space="PSUM") as ps_small,
            tc.tile_pool(name="ps_big", bufs=1, space="PSUM") as ps_big,
            tc.tile_pool(name="ps_y", bufs=2, space="PSUM") as ps_y,
        ):
            for blk in range(npc // 16):
                # one full PSUM bank accumulates y for 16 samples (8 rows each)
                y16 = ps_y.tile([128, D], F32, tag="y16")
                for j in range(8):  # pairs of samples
                    n0 = blk * 16 + 2 * j
                    z = z_bufs[(blk * 8 + j) % NZB]
                    nc.gpsimd.dma_start(
                        out=z[0:16, :],
                        in_=x_in[n0 : n0 + 2].rearrange("n s d -> (n s) d"),
                    )

                    # paired S1 + S2 diffs in one PSUM tile at base 0 (f32r
                    # matmuls must write base 0); relu ops write the Z rows
                    # cross-base (32-aligned bases only).
                    d24 = ps_small.tile([36, D], F32, tag="d24")
                    mm(d24[0:8, :], ct["a1"], z[0:16, :])
                    nc.vector.tensor_scalar(
                        out=z[32:40, :], in0=d24[0:8, :], scalar1=0.0, scalar2=None,
                        op0=AluOp.max,
                    )
                    mm(d24[0:36, :], ct["a2"], z[0:40, :])
                    nc.scalar.activation(out=z[64:100, :], in_=d24[0:36, :], func=Act.Relu)

                    # software-pipeline within the pair: all S3 matmuls and
                    # the (fused, pair-wide) relus first, then relu-independent
                    # c030v matmuls, then the relu-consuming y accumulations.
                    p8a_pair = p8a_pairs[j % len(p8a_pairs)]
                    p8b_pair = p8b_pairs[j % len(p8b_pairs)]
                    w8a = wpool.tile([128, 2 * D], mm_dtype, tag="w8a")
                    w8b = wpool.tile([97, 2 * D], mm_dtype, tag="w8b")
                    for par in range(2):
                        cs = slice(par * D, par * D + D)
                        mm(p8a_pair[:, cs], ct[f"a3a{par}"], z[:])
                        mm(p8b_pair[:, cs], ct[f"a3b{par}"], z[:])
                    nc.scalar.activation(out=w8a[:], in_=p8a_pair[:, :], func=Act.Relu)
                    nc.vector.tensor_scalar(
                        out=w8b[:], in0=p8b_pair[:, :], scalar1=0.0, scalar2=None,
                        op0=AluOp.max,
                    )
                    for par in range(2):
                        k = 2 * j + par
                        mm(y16[:], ct[f"c030v{k}"], z[:], start=(k == 0), stop=False)
                    for par in range(2):
                        k = 2 * j + par
                        cs = slice(par * D, par * D + D)
                        mm(y16[:], ct[f"c8av{k}"], w8a[:, cs], start=False, stop=False)
                        mm(y16[:], ct[f"c8bv{k}"], w8b[:, cs], start=False, stop=(k == 15))

                # ---- LayerNorm over (H, D) per sample + PReLU, batched ----
                r = lnpool.tile([128, 2], F32, tag="r")
                sq = lnpool.tile([128, D], F32, tag="sq")
                nc.vector.tensor_reduce(
                    out=r[:, 0:1], in_=y16[:], axis=mybir.AxisListType.X, op=AluOp.add
                )
                nc.scalar.activation(
                    out=sq[:], in_=y16[:], func=Act.Square, accum_out=r[:, 1:2]
                )
                sp = ps_small.tile([128, 2], F32, tag="d24")
                nc.tensor.matmul(sp[:], ct["g16"], r[:])

                mean = lnpool.tile([128, 1], F32, tag="mean")
                var = lnpool.tile([128, 1], F32, tag="var")
                rstd = lnpool.tile([128, 1], F32, tag="rstd")
                nc.vector.tensor_scalar(
                    out=mean[:], in0=sp[:, 0:1], scalar1=1.0 / (H * D), scalar2=None,
                    op0=AluOp.mult,
                )
                # var = sp[:,1]/HD - mean^2  (guard: compute mean^2 first)
                m2t = lnpool.tile([128, 1], F32, tag="m2t")
                nc.vector.tensor_tensor(
                    out=m2t[:], in0=mean[:], in1=mean[:], op=AluOp.mult
                )
                nc.vector.scalar_tensor_tensor(
                    out=var[:], in0=sp[:, 1:2], scalar=1.0 / (H * D),
                    in1=m2t[:], op0=AluOp.mult, op1=AluOp.subtract,
                )
                # rstd = 1/sqrt(var + eps)
                nc.scalar.activation(out=rstd[:], in_=var[:], func=Act.Sqrt, bias=eps)
                nc.vector.reciprocal(out=rstd[:], in_=rstd[:])

                t1 = lnpool.tile([128, D], F32, tag="t1")
                nc.vector.tensor_scalar(
                    out=t1[:], in0=y16[:], scalar1=mean[:], scalar2=rstd[:],
                    op0=AluOp.subtract, op1=AluOp.mult,
                )
                t2 = lnpool.tile([128, D], F32, tag="t2")
                nc.gpsimd.tensor_tensor(out=t2[:], in0=t1[:], in1=ct["lnw"], op=AluOp.mult)
                nc.gpsimd.tensor_tensor(out=t2[:], in0=t2[:], in1=ct["lnb"], op=AluOp.add)
                # prelu: out = max(t2, 0) + pre * min(t2, 0)
                u = lnpool.tile([128, D], F32, tag="u")
                nc.vector.tensor_scalar(
                    out=u[:], in0=t2[:], scalar1=0.0, scalar2=pre,
                    op0=AluOp.min, op1=AluOp.mult,
                )
                o16 = lnpool.tile([128, D], F32, tag="o16")
                nc.vector.scalar_tensor_tensor(
                    out=o16[:], in0=t2[:], scalar=0.0, in1=u[:],
                    op0=AluOp.max, op1=AluOp.add,
                )
                nc.gpsimd.dma_start(
                    out=y_out[blk * 16 : blk * 16 + 16].rearrange("n h d -> (n h) d"),
                    in_=o16[:],
                )

    nc.compile()
    return nc


# --------------------------------------------------------------------------
# Entry point
# --------------------------------------------------------------------------
_CACHED = {}


def _get_module():
    if "nc" not in _CACHED:
        _CACHED["nc"] = build_module()
    return _CACHED["nc"]


def host_feeds(FM, ln_weight, ln_bias, prelu_w):
    mats = _host_matrices(np.asarray(FM, np.float64))
    lnw = np.tile(np.asarray(ln_weight, np.float32).reshape(1, H, D), (16, 1, 1)).reshape(128, D)
    lnb = np.tile(np.asarray(ln_bias, np.float32).reshape(1, H, D), (16, 1, 1)).reshape(128, D)
    return _pack_consts(mats, lnw, lnb, float(np.asarray(prelu_w).reshape(-1)[0]))


def kernel(x, FM, ln_weight, ln_bias, prelu_w):
    x = np.ascontiguousarray(np.asarray(x, np.float32))
    cpack = np.ascontiguousarray(host_feeds(FM, ln_weight, ln_bias, prelu_w))

    nc = _get_module()
    in_maps = []
    for c in range(NCORES):
        in_maps.append(
            {"x": np.ascontiguousarray(x[c * NPC : (c + 1) * NPC]), "cpack": cpack}
        )

    res = run_bass_kernel_spmd(nc, in_maps, core_ids=list(range(NCORES)))
    out = np.concatenate([r["y"] for r in res.results], axis=0)
    return out.astype(np.float32)


if __name__ == "__main__":
    # smoke-test build
    nc = build_module(npc=16)
    print("module built ok")
